# revision 21
# baseline (speedup 1.0000x reference)
"""Trainium2 Bass kernel for per-sample-LoRA self-attention (non-causal SDPA).

Sharding: 8 cores = (batch b in 0..3) x (channel-half in 0..1).
LoRA deltas are merged into per-sample weights on the host (the sharding
hint's "merged weights"), so the device runs pure dense GEMMs. Each core
computes q/k/v for its 1024 output channels (8 heads) of sample b, runs
attention for those heads, and produces a partial output projection
(contraction over its half of the y channels). Host sums the two partials
per sample and transposes back.

All matmuls run as float32r (TF32-like, full PE rate at moving>=256).
Softmax denominators are column sums done as vector adds (off the PE)
plus one ones-matmul per chunk.
"""

import os
import sys

sys.path.insert(0, "/opt/trn_rl_repo")

import numpy as np

import concourse.bass as bass  # noqa: F401
import concourse.mybir as mybir
import concourse.tile as tile
from concourse import bacc, bass_utils

F32 = mybir.dt.float32
F32R = mybir.dt.float32r
BF16 = mybir.dt.bfloat16
AF = mybir.ActivationFunctionType

B, T, C = 4, 1024, 2048
H, D, R = 16, 128, 16
HALF = C // 2          # output channels per core
HH = HALF // D         # heads per core = 8
CT = C // 128          # contraction tiles over C = 16
IT = HALF // 128       # contraction tiles over half = 8
CH = 512               # t/free chunk
NCH = T // CH          # = 2
PTP = 2                # s_tiles per pT part
SCALE = 1.0 / float(np.sqrt(D))
ROPE_BASE = 10000.0

_compiled = {}
last_result = None     # BassKernelResults of the most recent run (for test harness)
PHASES = []            # (label, first instruction number) build-time markers


def _mark(nc, label):
    PHASES.append((label, int(nc.get_next_instruction_name().split("-")[1])))


def _build_nc():
    nc = bacc.Bacc("TRN2", target_bir_lowering=False, debug=False, num_devices=8)

    # Host-tiled layouts: partition dim first, per-partition rows contiguous.
    xt = nc.dram_tensor("xt", [128, CT, T], F32R, kind="ExternalInput").ap()
    Wqh = nc.dram_tensor("Wqh", [HH * 2, 128, CT // 2, D], F32R,
                         kind="ExternalInput").ap()
    Wkh = nc.dram_tensor("Wkh", [HH * 2, 128, CT // 2, D], F32R,
                         kind="ExternalInput").ap()
    Wvt = nc.dram_tensor("Wvt", [NCH, 128, CT, CH], F32R,
                         kind="ExternalInput").ap()
    Wot = nc.dram_tensor("Wot", [C // 128, 128, IT, 128], BF16,
                         kind="ExternalInput").ap()
    cosT = nc.dram_tensor("cosT", [D, T], F32, kind="ExternalInput").ap()
    sinTs = nc.dram_tensor("sinTs", [D, T], F32, kind="ExternalInput").ap()
    outT = nc.dram_tensor("outT", [C, T], F32, kind="ExternalOutput").ap()

    with tile.TileContext(nc) as tc:
        with tc.tile_pool(name="tabs", bufs=1) as tabs, \
             tc.tile_pool(name="ps_acc", bufs=2, space="PSUM") as ps_acc, \
             tc.tile_pool(name="ps_s", bufs=2, space="PSUM") as ps_s, \
             tc.tile_pool(name="ps_y", bufs=2, space="PSUM") as ps_y, \
             tc.tile_pool(name="ps_lb", bufs=2, space="PSUM") as ps_lb:

            _mark(nc, 'init')
            # ---------------- resident tables ----------------
            v_sb = tabs.tile([128, IT, HALF], BF16)   # [t_in_tile, t_tile, vo]
            y_sb = tabs.tile([128, HH, T], BF16)      # [d_in_tile, head, t]
            cos_sb = tabs.tile([D, T], F32)
            sin_sb = tabs.tile([D, T], F32)
            const_f = tabs.tile([128, 129], F32)
            const_r = tabs.tile([128, 129], F32R)
            ones128 = const_r[:, 0:1]

            # ============ phase 1: v, per-head qk+attention ============
            with tc.tile_pool(name="xpool", bufs=1) as xpool, \
                 tc.tile_pool(name="wqk", bufs=6) as wqkp:
                x_sb = xpool.tile([128, CT, T], F32R)
                pre_slabs = {}

                def issue_slabs(h):
                    for pi, wT in enumerate((Wqh, Wkh)):
                        for wh in range(2):
                            ws = wqkp.tile([128, CT // 2, 128], F32R,
                                           tag="wqk", name=f"ws{h}_{pi}{wh}")
                            nc.scalar.dma_start(ws[:], wT[h * 2 + wh])
                            pre_slabs[(h, pi, wh)] = ws

                _mark(nc, 'v')
                # PE warmup on the const tile while the first DMAs land
                nc.gpsimd.memset(const_f[:], 1.0)
                nc.vector.tensor_copy(const_r[:], const_f[:])
                for wi in range(12):
                    wps = ps_s.tile([128, CH], F32, tag="s")
                    nc.tensor.matmul(wps[:, 0:129], const_f[:, 1:129],
                                     const_f[:], start=True, stop=True)

                # ---- v = x @ Wv-half : 8 concurrent PSUM chains over tt,
                # x/wv DMAs interleaved per-ct so PE starts immediately ----
                with tc.tile_pool(name="wv", bufs=2) as wvp:
                    vpools = [ps_acc, ps_acc, ps_s, ps_s, ps_y, ps_y,
                              ps_lb, ps_lb]
                    vtags = ["acc", "acc", "s", "s", "y", "y", "lb", "lb"]
                    for ci in range(NCH):                # vo chunk of 512
                        wv = wvp.tile([128, CT, CH], F32R, tag="wv")
                        for ct in range(CT):
                            if ci == 0:
                                if ct == 0:
                                    # halved first slices: chains start sooner
                                    nc.sync.dma_start(x_sb[:, 0, 0:CH],
                                                      xt[:, 0, 0:CH])
                                    nc.scalar.dma_start(wv[:, 0, 0:256],
                                                        Wvt[0, :, 0, 0:256])
                                    nc.sync.dma_start(x_sb[:, 0, CH:T],
                                                      xt[:, 0, CH:T])
                                    nc.scalar.dma_start(wv[:, 0, 256:CH],
                                                        Wvt[0, :, 0, 256:CH])
                                    continue
                                nc.sync.dma_start(x_sb[:, ct, :], xt[:, ct, :])
                            nc.scalar.dma_start(wv[:, ct, :], Wvt[ci, :, ct, :])
                        if ci == 0:
                            nc.sync.dma_start(cos_sb[:], cosT[:])
                            nc.sync.dma_start(sin_sb[:], sinTs[:])
                        else:
                            issue_slabs(0)   # after ci=1's wv DMAs are queued
                        chains = [vpools[tt].tile([128, CH], F32,
                                                  tag=vtags[tt],
                                                  name=f"vch{ci}_{tt}")
                                  for tt in range(IT)]
                        for ct in range(CT):
                            for tt in range(IT):
                                nc.tensor.matmul(
                                    chains[tt][:],
                                    x_sb[:, ct, tt * 128:(tt + 1) * 128],
                                    wv[:, ct, :],
                                    start=(ct == 0), stop=(ct == CT - 1))
                        for tt in range(IT):
                            nc.scalar.activation(
                                v_sb[:, tt, ci * CH:(ci + 1) * CH],
                                chains[tt][:], AF.Copy)

                # ---- per-head: qk projection + RoPE + attention ----
                with tc.tile_pool(name="rope", bufs=2) as rope, \
                     tc.tile_pool(name="qk", bufs=3) as qkp, \
                     tc.tile_pool(name="ptp", bufs=5) as ptp, \
                     tc.tile_pool(name="att", bufs=1) as att, \
                     tc.tile_pool(name="wo", bufs=3) as wop, \
                     tc.tile_pool(name="outp", bufs=3) as outp:
                    p2_pre = {}
                    for h in range(HH):
                        _mark(nc, f'qk{h}')
                        rots = []
                        for pi, wT in enumerate((Wqh, Wkh)):
                            rot = qkp.tile([D, T], F32R, tag="rot")
                            slabs = []
                            for wh in range(2):
                                if (h, pi, wh) in pre_slabs:
                                    slabs.append(pre_slabs.pop((h, pi, wh)))
                                    continue
                                ws = wqkp.tile([128, CT // 2, 128], F32R,
                                               tag="wqk")
                                nc.scalar.dma_start(ws[:], wT[h * 2 + wh])
                                slabs.append(ws)
                            for ci in range(NCH):
                                ps = ps_acc.tile([128, CH], F32, tag="acc")
                                for ct in range(CT):
                                    nc.tensor.matmul(
                                        ps[:],
                                        slabs[ct // (CT // 2)][:, ct % (CT // 2), :],
                                        x_sb[:, ct, ci * CH:(ci + 1) * CH],
                                        start=(ct == 0), stop=(ct == CT - 1))
                                # RoPE: PSUM -> SBUF copy, shift, mul, add
                                q0 = rope.tile([D, CH], F32, tag="q0")
                                nc.vector.tensor_copy(q0[:], ps[:])
                                sh = rope.tile([D, CH], F32, tag="sh")
                                nc.sync.dma_start(sh[0:64, :], q0[64:128, :])
                                nc.sync.dma_start(sh[64:128, :], q0[0:64, :])
                                nc.vector.tensor_mul(sh[:], sh[:],
                                                     sin_sb[:, ci * CH:(ci + 1) * CH])
                                nc.vector.tensor_mul(q0[:], q0[:],
                                                     cos_sb[:, ci * CH:(ci + 1) * CH])
                                nc.vector.tensor_add(rot[:, ci * CH:(ci + 1) * CH],
                                                     q0[:], sh[:])
                            rots.append(rot)
                        qr, kr = rots

                        _mark(nc, f'a1_{h}')
                        # A1 (2-bank psum per s_tile) + exp (bf16); l
                        # accumulated as a bf16 tree on Vector
                        t01 = att.tile([128, PTP, T], BF16, tag="t01")
                        t23 = att.tile([128, PTP, T], BF16, tag="t23")
                        l_sum = att.tile([128, T], F32R, tag="l_sum")
                        pts = []
                        for part in range(IT // PTP):
                            pT = ptp.tile([128, PTP, T], BF16, tag="pT")
                            for sp in range(PTP):
                                st = part * PTP + sp
                                for ci in range(NCH):
                                    ps = ps_s.tile([128, CH], F32, tag="s")
                                    nc.tensor.matmul(ps[:],
                                                     kr[:, st * 128:(st + 1) * 128],
                                                     qr[:, ci * CH:(ci + 1) * CH],
                                                     start=True, stop=True)
                                    nc.scalar.activation(pT[:, sp, ci * CH:(ci + 1) * CH],
                                                         ps[:], AF.Exp, scale=SCALE)
                            pts.append(pT)
                            if part == 1:
                                nc.vector.tensor_add(t01[:], pts[0][:], pts[1][:])
                            elif part == 3:
                                nc.vector.tensor_add(t23[:], pts[2][:], pts[3][:])

                        _mark(nc, f'a2_{h}')
                        # A2 into PSUM (l matmuls deferred until after these
                        # so the PE never waits on the Vector tree)
                        yps = []
                        for ci in range(NCH):
                            yp = ps_y.tile([D, CH], F32, tag="y")
                            for st in range(IT):
                                nc.tensor.matmul(yp[:], v_sb[:, st, h * D:(h + 1) * D],
                                                 pts[st // PTP][:, st % PTP,
                                                                ci * CH:(ci + 1) * CH],
                                                 start=(st == 0), stop=(st == IT - 1))
                            yps.append(yp)

                        if h == HH - 1:
                            # prefetch wo and run ot=0's first 7 it-steps so
                            # the PE has work while head-7's l tree finishes
                            for oti in range(3):
                                wo = wop.tile([128, IT, 128], BF16, tag="wo",
                                              name=f"wo_pre{oti}")
                                nc.scalar.dma_start(wo[:], Wot[oti])
                                p2_pre[oti] = wo
                            ot0_chains = []
                            for ci in range(NCH):
                                ps = ps_acc.tile([128, CH], F32, tag="acc",
                                                 name=f"p2pre{ci}")
                                for it in range(IT - 1):
                                    nc.tensor.matmul(
                                        ps[:], p2_pre[0][:, it, :],
                                        y_sb[:, it, ci * CH:(ci + 1) * CH],
                                        start=(it == 0), stop=False)
                                ot0_chains.append(ps)

                        _mark(nc, f'l_{h}')
                        # finish the l tree, then partition-sum, reciprocal
                        # on one partition, GpSimd broadcast to 128
                        nc.vector.tensor_add(t01[:], t01[:], t23[:])
                        nc.vector.tensor_add(l_sum[:], t01[:, 0, :], t01[:, 1, :])
                        rb = att.tile([128, T], F32, tag="rb")
                        r_row = att.tile([1, T], F32, tag="r_row")
                        for ci in range(NCH):
                            l_ps = ps_lb.tile([1, CH], F32, tag="lb")
                            nc.tensor.matmul(l_ps[:], ones128,
                                             l_sum[:, ci * CH:(ci + 1) * CH],
                                             start=True, stop=True)
                            nc.vector.reciprocal_approx_fast(
                                out=r_row[:, ci * CH:(ci + 1) * CH], in_=l_ps[:])
                            nc.gpsimd.partition_broadcast(
                                rb[:, ci * CH:(ci + 1) * CH],
                                r_row[:, ci * CH:(ci + 1) * CH])
                            nc.vector.tensor_mul(y_sb[:, h, ci * CH:(ci + 1) * CH],
                                                 yps[ci][:],
                                                 rb[:, ci * CH:(ci + 1) * CH])

                    # ======== phase 2: out^T = Wo-half contraction ========
                    _mark(nc, 'p2')
                    for ot in range(C // 128):
                        if ot == 0:
                            wo = p2_pre[0]
                            for ci in range(NCH):
                                ps = ot0_chains[ci]
                                nc.tensor.matmul(
                                    ps[:], wo[:, IT - 1, :],
                                    y_sb[:, IT - 1, ci * CH:(ci + 1) * CH],
                                    start=False, stop=True)
                                o_sb = outp.tile([128, CH], F32, tag="o")
                                nc.scalar.activation(o_sb[:], ps[:], AF.Copy)
                                nc.sync.dma_start(
                                    outT[0:128, ci * CH:(ci + 1) * CH], o_sb[:])
                            continue
                        if ot in p2_pre:
                            wo = p2_pre[ot]
                        else:
                            wo = wop.tile([128, IT, 128], BF16, tag="wo")
                            nc.scalar.dma_start(wo[:], Wot[ot])
                        for ci in range(NCH):
                            ps = ps_acc.tile([128, CH], F32, tag="acc")
                            for it in range(IT):
                                nc.tensor.matmul(ps[:], wo[:, it, :],
                                                 y_sb[:, it, ci * CH:(ci + 1) * CH],
                                                 start=(it == 0), stop=(it == IT - 1))
                            o_sb = outp.tile([128, CH], F32, tag="o")
                            nc.scalar.activation(o_sb[:], ps[:], AF.Copy)
                            nc.sync.dma_start(outT[ot * 128:(ot + 1) * 128,
                                                   ci * CH:(ci + 1) * CH], o_sb[:])

    nc.compile()
    return nc


def _rope_tables():
    inv = (1.0 / (ROPE_BASE ** (np.arange(0, D, 2, dtype=np.float32) / np.float32(D)))).astype(np.float32)
    t_ar = np.arange(T, dtype=np.float32)
    fr = t_ar[:, None] * inv[None, :]
    emb = np.concatenate([fr, fr], axis=1)          # [T, D]
    cos = np.cos(emb).astype(np.float32).T.copy()   # [D, T]
    sin = np.sin(emb).astype(np.float32).T.copy()
    sins = sin.copy()
    sins[:64, :] *= -1.0
    return np.ascontiguousarray(cos), np.ascontiguousarray(sins)


def kernel(x, qkvo_delta, Wq, Wk, Wv, Wo):
    global last_result
    x = np.asarray(x, dtype=np.float32)
    qkvo_delta = np.asarray(qkvo_delta, dtype=np.float32)
    Wq = np.asarray(Wq, dtype=np.float32)
    Wk = np.asarray(Wk, dtype=np.float32)
    Wv = np.asarray(Wv, dtype=np.float32)
    Wo = np.asarray(Wo, dtype=np.float32)

    if "nc" not in _compiled:
        _compiled["nc"] = _build_nc()
    nc = _compiled["nc"]

    cos, sins = _rope_tables()
    d = qkvo_delta.reshape(B, 8, R, C)
    dqA, dqB, dkA, dkB, dvA, dvB, doA, doB = (d[:, i] for i in range(8))

    def tile_qk(WT):
        # WT [C, HALF] -> [HH*2, 128, CT//2, D] (head-slab, partition-major)
        a = WT.reshape(2, CT // 2, 128, HH, D)       # [wh, ct2, p, h, d]
        return np.ascontiguousarray(
            a.transpose(3, 0, 2, 1, 4).reshape(HH * 2, 128, CT // 2, D))

    in_maps = []
    for core in range(8):
        b, half = core // 2, core % 2
        sl = slice(half * HALF, (half + 1) * HALF)
        # host-merged per-sample weights (W + dB^T @ dA)
        Wq_m = Wq + dqB[b].T @ dqA[b]
        Wk_m = Wk + dkB[b].T @ dkA[b]
        Wv_m = Wv + dvB[b].T @ dvA[b]
        Wo_m = Wo + doB[b].T @ doA[b]

        WqT = Wq_m[sl, :].T                          # [C, HALF]
        WkT = Wk_m[sl, :].T
        WvT = Wv_m[sl, :].T                          # [C, HALF]
        WoT = Wo_m[:, sl].T                          # [HALF, C]

        xtile = np.ascontiguousarray(
            x[b].T.reshape(CT, 128, T).transpose(1, 0, 2))   # [128, CT, T]
        wvt = np.ascontiguousarray(
            WvT.reshape(CT, 128, NCH, CH).transpose(2, 1, 0, 3))  # [ci,128,CT,CH]
        wot = np.ascontiguousarray(
            WoT.reshape(IT, 128, C // 128, 128).transpose(2, 1, 0, 3)
            .astype(mybir.dt.np(BF16)))                     # [ot,128,IT,128] bf16

        in_maps.append({
            "xt": xtile,
            "Wqh": tile_qk(WqT),
            "Wkh": tile_qk(WkT),
            "Wvt": wvt,
            "Wot": wot,
            "cosT": cos,
            "sinTs": sins,
        })

    trace = bool(int(os.environ.get("KERNEL_TRACE", "0")))
    res = bass_utils.run_bass_kernel_spmd(
        nc, in_maps, core_ids=list(range(8)), trace=trace)
    last_result = res

    out = np.empty((B, T, C), dtype=np.float32)
    for b in range(B):
        acc = res.results[2 * b]["outT"].astype(np.float32) + \
            res.results[2 * b + 1]["outT"].astype(np.float32)
        out[b] = acc.T
    return out
